# revision 3
# baseline (speedup 1.0000x reference)
"""Chebyshev positional-embedding expansion kernel for Trainium2 (8 cores).

Computes out[b, s, d] = T_d(xhat[b, s]), xhat = 2*input_ids/max_seq_len - 1,
T_d = Chebyshev polynomial of the first kind, matching the jax.lax.scan
reference recurrence T_n = 2*xhat*T_{n-1} - T_{n-2} to ~1e-4 rel error.

Strategy (per core; batch row b == core id, no communication):
  T_d(cos t) = cos(d*t), so with phi = arccos(xhat)/(2pi) in [0, 0.5]:
      T[s, n] = cos(2pi * n * phi_s) = sin(2pi * cfrac(n*phi_s + 0.25))
  where cfrac(x) = x - round(x). Per position build 64-entry tables
      c_b = cfrac(b*phi)           b = n % 32
      g_a = cfrac(32a*phi) + 0.25  a = n // 32
  (table phases via PE matmuls + DVE round-cast). Then per 128-position
  block and hh pair, ONE K=64 PE matmul with a constant one-hot selector
  produces y[s, n] = c_{n%32} + g_{n//32} in (-1.25, 1.75); one DVE
  ADD_RANGE_WRAP custom op folds it to [-0.5, 0.5]; one ACT Sin
  (scale=2pi) gives T. Per output element: 1 PE col-cycle + 1 DVE op +
  1 ACT op -- every engine stays under the ~47us/core HBM write roofline,
  so the kernel is output-DMA bound.

Layout: s_local = p*32 + hh -> partition p owns 32 contiguous output rows;
output streamed to HBM in 9 chunks (1-2-2-2-2-2-2-2-1 hh-pairs) on the
sync-engine HWDGE queue while later pairs compute.
"""

import numpy as np

import concourse.bacc as bacc
import concourse.mybir as mybir
from concourse import tile
from concourse.bass_utils import run_bass_kernel_spmd

F32 = mybir.dt.float32
I32 = mybir.dt.int32
U32 = mybir.dt.uint32
OP = mybir.AluOpType
AF = mybir.ActivationFunctionType

N_CORES = 8
B, S, D = 8, 4096, 1024
MAX_SEQ_LEN = 4096
S_PER = B * S // N_CORES
P = 128
H = S_PER // P  # 32
PI = float(np.pi)

# output chunking in hh-PAIRS (sum = 16 pairs = 32 hh)
CHUNK_PAIRS = (1, 2, 2, 2, 2, 2, 2, 2, 1)


def _emit_body(nc, tc, sb, out_pool, x2d, out3):
    # ---------------- constants (no input dependency) ----------------
    ONES = sb.tile([P, P], F32, tag="ONES")
    IDN = sb.tile([P, P], F32, tag="IDN")
    nc.gpsimd.memset(ONES[:], 1.0)
    nc.gpsimd.affine_select(IDN[:], ONES[:], [[1, P]], OP.is_equal, 0.0,
                            base=0, channel_multiplier=-1)

    # block-diag ramp BD[k, hh*32+b] = b if hh == k%32 else 0   [64, 1024]
    BD = sb.tile([64, H * 32], F32, tag="BD")
    nc.gpsimd.iota(BD[:], [[0, 32], [1, 32]], base=0, channel_multiplier=0,
                   allow_small_or_imprecise_dtypes=True)
    BD3 = BD[:].rearrange("k (h b) -> k h b", h=H)
    nc.gpsimd.affine_select(BD3[0:32], BD3[0:32], [[1, 32], [0, 32]],
                            OP.is_equal, 0.0, base=0, channel_multiplier=-1)
    nc.gpsimd.affine_select(BD3[32:64], BD3[32:64], [[1, 32], [0, 32]],
                            OP.is_equal, 0.0, base=0, channel_multiplier=-1)

    # one-hot selector [128, 1024], rows 0:64 == rows 64:128:
    #   row u (u<32):  1 where n%32 == u ;  row 32+a: 1 where n//32 == a
    SEL = sb.tile([P, D], F32, tag="SEL")
    nc.gpsimd.memset(SEL[:], 1.0)
    SEL4 = SEL[:].rearrange("u (a b) -> u a b", a=32)
    for half in (0, 64):
        nc.gpsimd.affine_select(SEL4[half:half + 32], SEL4[half:half + 32],
                                [[0, 32], [1, 32]], OP.is_equal, 0.0,
                                base=0, channel_multiplier=-1)
        nc.gpsimd.affine_select(SEL4[half + 32:half + 64],
                                SEL4[half + 32:half + 64],
                                [[1, 32], [0, 32]], OP.is_equal, 0.0,
                                base=0, channel_multiplier=-1)

    # ---------------- input + phi = arccos(xhat)/(2pi) ----------------
    X = sb.tile([P, H], F32, tag="X")
    nc.sync.dma_start(X[:], x2d)

    XH = sb.tile([P, H], F32, tag="XH")
    A1 = sb.tile([P, H], F32, tag="A1")
    B1 = sb.tile([P, H], F32, tag="B1")
    S2 = sb.tile([P, H], F32, tag="S2")
    R = sb.tile([P, H], F32, tag="R")
    AX = sb.tile([P, H], F32, tag="AX")
    M = sb.tile([P, H], F32, tag="M")
    MI = sb.tile([P, H], I32, tag="MI")
    NEG = sb.tile([P, H], F32, tag="NEG")
    DEN = sb.tile([P, H], F32, tag="DEN")
    RD = sb.tile([P, H], F32, tag="RD")
    NUM = sb.tile([P, H], F32, tag="NUM")
    T1 = sb.tile([P, H], F32, tag="T1")
    AT = sb.tile([P, H], F32, tag="AT")
    SS = sb.tile([P, H], F32, tag="SS")
    AA = sb.tile([P, H], F32, tag="AA")
    CH = sb.tile([P, H], F32, tag="CH")
    TH = sb.tile([P, H], F32, tag="TH")
    PHI = sb.tile([P, 64], F32, tag="PHI")  # cols 0:32 phi, 32:64 psi=32phi

    nc.vector.tensor_scalar(XH[:], X[:], 1.0 / (MAX_SEQ_LEN / 2), -1.0,
                            OP.mult, OP.add)
    nc.gpsimd.tensor_scalar(A1[:], XH[:], -1.0, 1.0, OP.mult, OP.add)
    nc.gpsimd.tensor_scalar(B1[:], XH[:], 1.0, 1.0, OP.mult, OP.add)
    nc.gpsimd.tensor_tensor(S2[:], A1[:], B1[:], OP.mult)
    nc.scalar.activation(R[:], S2[:], AF.Sqrt)
    nc.vector.tensor_scalar(AX[:].bitcast(U32), XH[:].bitcast(U32),
                            0x7FFFFFFF, None, OP.bitwise_and)
    nc.gpsimd.tensor_scalar(M[:], AX[:], 0.70710678, None, OP.is_le)
    nc.vector.tensor_copy(MI[:], M[:])
    nc.gpsimd.tensor_scalar(NEG[:], XH[:], 0.0, None, OP.is_lt)
    nc.vector.tensor_tensor(DEN[:], R[:], AX[:], OP.max)
    nc.vector.reciprocal(RD[:], DEN[:])
    nc.gpsimd.tensor_copy(NUM[:], R[:])
    nc.vector.copy_predicated(NUM[:], MI[:], XH[:])
    nc.vector.tensor_tensor(T1[:], NUM[:], RD[:], OP.mult)
    nc.scalar.activation(AT[:], T1[:], AF.Arctan)
    # theta = (1-2m)*at + (m ? pi/2 : neg*pi)
    nc.gpsimd.tensor_scalar(SS[:], M[:], -2.0, 1.0, OP.mult, OP.add)
    nc.gpsimd.tensor_scalar(AA[:], NEG[:], PI, None, OP.mult)
    nc.gpsimd.memset(CH[:], PI / 2)
    nc.vector.copy_predicated(AA[:], MI[:], CH[:])
    nc.vector.tensor_tensor(TH[:], AT[:], SS[:], OP.mult)
    nc.vector.tensor_tensor(TH[:], TH[:], AA[:], OP.add)
    nc.vector.tensor_scalar(PHI[:, 0:H], TH[:], 1.0 / (2 * PI), None,
                            OP.mult)
    nc.vector.tensor_scalar(PHI[:, H:2 * H], PHI[:, 0:H], 32.0, None,
                            OP.mult)

    # ---------------- tables ----------------
    TBL = sb.tile([P, H, 64], F32, tag="TBL")
    TBLT = sb.tile([P, 16 * P], F32, tag="TBLT")

    with tc.tile_pool(name="psA", bufs=1, space="PSUM") as psA:
        PHT_PS = psA.tile([64, P], F32, tag="PHT_PS")
        nc.tensor.transpose(PHT_PS[:], PHI[:], IDN[:])
        PHT = sb.tile([64, P], F32, tag="PHT")
        nc.scalar.activation(PHT[:], PHT_PS[:], AF.Copy)

        YC = psA.tile([P, H * 32], F32, tag="YC")
        YG = psA.tile([P, H * 32], F32, tag="YG")
        for j in (0, 1):
            nc.tensor.matmul(YC[:, j * 512:(j + 1) * 512], PHT[0:32, :],
                             BD[0:32, j * 512:(j + 1) * 512])
            nc.tensor.matmul(YG[:, j * 512:(j + 1) * 512], PHT[32:64, :],
                             BD[32:64, j * 512:(j + 1) * 512])

        # c = y - round(y); g = (y + 0.25) - round(y)
        KC = sb.tile([P, H * 32], I32, tag="KC")
        KG = sb.tile([P, H * 32], I32, tag="KG")
        nc.vector.tensor_copy(KC[:], YC[:])
        nc.vector.tensor_copy(KG[:], YG[:])
        YC3 = YC[:].rearrange("p (h b) -> p h b", h=H)
        YG3 = YG[:].rearrange("p (h a) -> p h a", h=H)
        KC3 = KC[:].rearrange("p (h b) -> p h b", h=H)
        KG3 = KG[:].rearrange("p (h a) -> p h a", h=H)
        nc.vector.scalar_tensor_tensor(TBL[:, :, 0:32], YC3[:, :, :], 0.0,
                                       KC3[:, :, :], OP.subtract,
                                       OP.subtract)
        nc.vector.scalar_tensor_tensor(TBL[:, :, 32:64], YG3[:, :, :],
                                       -0.25, KG3[:, :, :], OP.subtract,
                                       OP.subtract)

        # transpose TBL -> per-hh lhsT tables
        TBL2 = TBL[:].rearrange("p h u -> p (h u)")
        for j in range(16):
            TP = psA.tile([P, P], F32, tag=f"TP{j % 2}")
            nc.tensor.transpose(TP[:], TBL2[:, j * P:(j + 1) * P], IDN[:])
            nc.scalar.activation(TBLT[:, j * P:(j + 1) * P], TP[:], AF.Copy)

    # ---------------- big loop: hh pairs ----------------
    with tc.tile_pool(name="psB", bufs=1, space="PSUM") as psB:
        pair = 0
        for ci, npairs in enumerate(CHUNK_PAIRS):
            nh = npairs * 2
            OUT = out_pool.tile([P, nh, D], F32, tag=f"OUT{nh}")
            h0 = pair * 2
            for q in range(npairs):
                hh0 = h0 + q * 2  # first hh of this pair
                Y = psB.tile([P, 2 * D], F32, tag=f"Y{pair % 2}")
                for t in range(2):  # the two hh of the pair
                    hh = hh0 + t
                    lhsT = TBLT[(hh % 2) * 64:(hh % 2) * 64 + 64,
                                (hh // 2) * P:(hh // 2) * P + P]
                    rb = (hh % 2) * 64
                    nc.tensor.matmul(Y[:, t * D:t * D + 512], lhsT,
                                     SEL[rb:rb + 64, 0:512])
                    nc.tensor.matmul(Y[:, t * D + 512:t * D + 1024], lhsT,
                                     SEL[rb:rb + 64, 512:1024])
                nc.vector.add_range_wrap(Y[:], Y[:], 0.0, 0.5, 1.0)
                nc.scalar.activation(OUT[:, q * 2:q * 2 + 2, :], Y[:],
                                     AF.Sin, bias=0.0, scale=2 * PI)
                pair += 1
            nc.sync.dma_start(out3[:, h0:h0 + nh, :], OUT[:])


def build_nc():
    nc = bacc.Bacc("TRN2", target_bir_lowering=False, debug=False,
                   num_devices=N_CORES)
    x = nc.dram_tensor("x", [S_PER], F32, kind="ExternalInput")
    out = nc.dram_tensor("out", [S_PER, D], F32, kind="ExternalOutput")
    x2d = x.rearrange("(p h) -> p h", p=P)
    out3 = out.rearrange("(p h) d -> p h d", p=P)

    with tile.TileContext(nc) as tc:
        with (
            tc.tile_pool(name="sb", bufs=1) as sb,
            tc.tile_pool(name="outp", bufs=3) as out_pool,
        ):
            _emit_body(nc, tc, sb, out_pool, x2d, out3)

    nc.compile()
    return nc


_CACHED_NC = None


def kernel(input_ids, max_seq_len, d_model):
    """Full-input entry point: shards batch rows across the 8 cores."""
    global _CACHED_NC
    input_ids = np.ascontiguousarray(np.asarray(input_ids, dtype=np.float32))
    assert input_ids.shape == (B, S) and int(max_seq_len) == MAX_SEQ_LEN \
        and int(d_model) == D
    if _CACHED_NC is None:
        _CACHED_NC = build_nc()
    in_maps = [{"x": input_ids[c]} for c in range(N_CORES)]
    res = run_bass_kernel_spmd(_CACHED_NC, in_maps,
                               core_ids=list(range(N_CORES)))
    return np.stack([res.results[c]["out"] for c in range(N_CORES)], axis=0)


# revision 5
# speedup vs baseline: 1.2468x; 1.2468x over previous
"""Chebyshev positional-embedding expansion kernel for Trainium2 (8 cores).

Computes out[b, s, d] = T_d(xhat[b, s]), xhat = 2*input_ids/max_seq_len - 1,
T_d = Chebyshev polynomial of the first kind, matching the jax.lax.scan
reference recurrence T_n = 2*xhat*T_{n-1} - T_{n-2} to ~3e-3 rel error.

Strategy (per core; batch row b == core id, no communication):
  T_d(cos t) = cos(d*t), so with phi = arccos(xhat)/(2pi) in [0, 0.5]:
      T[s, n] = cos(2pi * n * phi_s) = sin(2pi * cfrac(n*phi_s + 0.25))
  where cfrac(x) = x - round(x). Per position build 64-entry f16 tables
      c_b = cfrac(b*phi)           b = n % 32
      g_a = cfrac(32a*phi) + 0.25  a = n // 32
  (table phases via two K=32 PE matmuls against a host-provided
  block-diagonal ramp; cfrac via DVE round-to-nearest int cast). A single
  2-byte DMA-xbar transpose turns the tables into per-hh [64, 128] lhsT
  blocks. Then per 128-position block and hh pair, K=64 f16 PE matmuls
  against a constant one-hot selector produce
      y[s, n] = c_{n%32} + g_{n//32} in (-1.25, 1.75)
  in PSUM; one DVE ADD_RANGE_WRAP custom op folds y to [-0.5, 0.5]; one
  ACT Sin (scale=2pi) writes T to SBUF. Per output element: 1 f16 PE
  col-cycle + 1 DVE op + 1 ACT op -- every engine stays under the
  ~47us/core HBM write roofline, so the kernel is output-DMA bound.

Constants (one-hot selector, block-diag ramp, transpose identity) are
precomputed on the host and DMA'd in on the gpsimd queue so the sync
HWDGE queue carries only the streamed output chunks.

Layout: s_local = p*32 + hh -> partition p owns 32 contiguous output rows.
"""

import numpy as np

import concourse.bacc as bacc
import concourse.mybir as mybir
from concourse import tile
from concourse.bass_utils import run_bass_kernel_spmd

F32 = mybir.dt.float32
F16 = mybir.dt.float16
I32 = mybir.dt.int32
U32 = mybir.dt.uint32
OP = mybir.AluOpType
AF = mybir.ActivationFunctionType

N_CORES = 8
B, S, D = 8, 4096, 1024
MAX_SEQ_LEN = 4096
S_PER = B * S // N_CORES
P = 128
H = S_PER // P  # 32
PI = float(np.pi)

# output chunk sizes in hh units (sum = 32); first/last small so the DMA
# stream starts early and drains fast
CHUNKS = (1, 1, 2, 4, 4, 4, 4, 4, 4, 4)


def host_constants():
    n = np.arange(D)
    sel = np.zeros((P, D), np.float16)
    sel[0:32] = (n[None, :] % 32 == np.arange(32)[:, None])
    sel[32:64] = (n[None, :] // 32 == np.arange(32)[:, None])
    sel[64:128] = sel[0:64]
    bd = np.zeros((64, H * 32), np.float32)
    for k in range(32):
        bd[k, k * 32:(k + 1) * 32] = np.arange(32)
        bd[32 + k, k * 32:(k + 1) * 32] = np.arange(32)
    idn = np.eye(P, dtype=np.float32)
    return {"sel": sel, "bd": bd, "idn": idn}


def _emit_body(nc, tc, sb, out_pool, x2d, out3, sel_d, bd_d, idn_d):
    # ---------------- constants via DMA (gpsimd SWDGE queue) ----------
    SEL = sb.tile([P, D], F16, tag="SEL")
    BD = sb.tile([64, H * 32], F32, tag="BD")
    IDN = sb.tile([P, P], F32, tag="IDN")
    nc.gpsimd.dma_start(SEL[:], sel_d)
    nc.gpsimd.dma_start(BD[:], bd_d)
    nc.gpsimd.dma_start(IDN[:], idn_d)

    # ---------------- input + phi = arccos(xhat)/(2pi) ----------------
    X = sb.tile([P, H], F32, tag="X")
    nc.gpsimd.dma_start(X[:], x2d)

    XH = sb.tile([P, H], F32, tag="XH")
    SQ = sb.tile([P, H], F32, tag="SQ")
    S2 = sb.tile([P, H], F32, tag="S2")
    R = sb.tile([P, H], F32, tag="R")
    AX = sb.tile([P, H], F32, tag="AX")
    M = sb.tile([P, H], F32, tag="M")
    MI = sb.tile([P, H], I32, tag="MI")
    NEG = sb.tile([P, H], F32, tag="NEG")
    DEN = sb.tile([P, H], F32, tag="DEN")
    RD = sb.tile([P, H], F32, tag="RD")
    NUM = sb.tile([P, H], F32, tag="NUM")
    T1 = sb.tile([P, H], F32, tag="T1")
    AT = sb.tile([P, H], F32, tag="AT")
    SS = sb.tile([P, H], F32, tag="SS")
    AA = sb.tile([P, H], F32, tag="AA")
    CH = sb.tile([P, H], F32, tag="CH")
    TH = sb.tile([P, H], F32, tag="TH")
    PHI = sb.tile([P, 64], F32, tag="PHI")  # cols 0:32 phi, 32:64 psi=32phi

    nc.vector.tensor_scalar(XH[:], X[:], 1.0 / (MAX_SEQ_LEN / 2), -1.0,
                            OP.mult, OP.add)
    nc.scalar.activation(SQ[:], XH[:], AF.Square)
    nc.vector.tensor_scalar(S2[:], SQ[:], -1.0, 1.0, OP.mult, OP.add)
    nc.scalar.activation(R[:], S2[:], AF.Sqrt)
    nc.vector.tensor_scalar(AX[:].bitcast(U32), XH[:].bitcast(U32),
                            0x7FFFFFFF, None, OP.bitwise_and)
    nc.gpsimd.tensor_scalar(M[:], AX[:], 0.70710678, None, OP.is_le)
    nc.vector.tensor_copy(MI[:], M[:])
    nc.gpsimd.tensor_scalar(NEG[:], XH[:], 0.0, None, OP.is_lt)
    nc.vector.tensor_tensor(DEN[:], R[:], AX[:], OP.max)
    nc.vector.reciprocal(RD[:], DEN[:])
    nc.gpsimd.tensor_copy(NUM[:], R[:])
    nc.vector.copy_predicated(NUM[:], MI[:], XH[:])
    nc.vector.tensor_tensor(T1[:], NUM[:], RD[:], OP.mult)
    nc.scalar.activation(AT[:], T1[:], AF.Arctan)
    # theta = (1-2m)*at + (m ? pi/2 : neg*pi)
    nc.gpsimd.tensor_scalar(SS[:], M[:], -2.0, 1.0, OP.mult, OP.add)
    nc.gpsimd.tensor_scalar(AA[:], NEG[:], PI, None, OP.mult)
    nc.gpsimd.memset(CH[:], PI / 2)
    nc.vector.copy_predicated(AA[:], MI[:], CH[:])
    nc.vector.tensor_tensor(TH[:], AT[:], SS[:], OP.mult)
    nc.vector.tensor_tensor(TH[:], TH[:], AA[:], OP.add)
    nc.vector.tensor_scalar(PHI[:, 0:H], TH[:], 1.0 / (2 * PI), None,
                            OP.mult)
    nc.vector.tensor_scalar(PHI[:, H:2 * H], PHI[:, 0:H], 32.0, None,
                            OP.mult)

    # ---------------- tables (f16) ----------------
    TBL = sb.tile([P, H, 64], F16, tag="TBL")
    TBLT = sb.tile([P, 16, P], F16, tag="TBLT")

    with tc.tile_pool(name="psA", bufs=1, space="PSUM") as psA:
        PHT_PS = psA.tile([64, P], F32, tag="PHT_PS")
        nc.tensor.transpose(PHT_PS[:], PHI[:], IDN[:])
        PHT = sb.tile([64, P], F32, tag="PHT")
        nc.scalar.activation(PHT[:], PHT_PS[:], AF.Copy)

        YC = psA.tile([P, H * 32], F32, tag="YC")
        YG = psA.tile([P, H * 32], F32, tag="YG")
        for j in (0, 1):
            nc.tensor.matmul(YC[:, j * 512:(j + 1) * 512], PHT[0:32, :],
                             BD[0:32, j * 512:(j + 1) * 512])
            nc.tensor.matmul(YG[:, j * 512:(j + 1) * 512], PHT[32:64, :],
                             BD[32:64, j * 512:(j + 1) * 512])

        # c = y - round(y); g = (y + 0.25) - round(y)   (round via i32 cast)
        KC = sb.tile([P, H * 32], I32, tag="KC")
        KG = sb.tile([P, H * 32], I32, tag="KG")
        nc.vector.tensor_copy(KC[:], YC[:])
        nc.vector.tensor_copy(KG[:], YG[:])
        YC3 = YC[:].rearrange("p (h b) -> p h b", h=H)
        YG3 = YG[:].rearrange("p (h a) -> p h a", h=H)
        KC3 = KC[:].rearrange("p (h b) -> p h b", h=H)
        KG3 = KG[:].rearrange("p (h a) -> p h a", h=H)
        nc.vector.scalar_tensor_tensor(TBL[:, :, 0:32], YC3[:, :, :], 0.0,
                                       KC3[:, :, :], OP.subtract,
                                       OP.subtract)
        nc.vector.scalar_tensor_tensor(TBL[:, :, 32:64], YG3[:, :, :],
                                       -0.25, KG3[:, :, :], OP.subtract,
                                       OP.subtract)

    # one 2-byte xbar transpose: TBLT[p, j, q] = TBL2[q, j*128+p]
    TBL2 = TBL[:].rearrange("p h u -> p (h u)")
    nc.scalar.dma_start_transpose(TBLT[:], TBL2)

    # ---------------- big loop: hh pairs ----------------
    with tc.tile_pool(name="psB", bufs=1, space="PSUM") as psB:

        def emit_hh(hh, Y, yoff):
            lhsT = TBLT[(hh % 2) * 64:(hh % 2) * 64 + 64, hh // 2, :]
            rb = (hh % 2) * 64
            nc.tensor.matmul(Y[:, yoff:yoff + 512], lhsT,
                             SEL[rb:rb + 64, 0:512])
            nc.tensor.matmul(Y[:, yoff + 512:yoff + 1024], lhsT,
                             SEL[rb:rb + 64, 512:1024])

        hh = 0
        for nh in CHUNKS:
            OUT = out_pool.tile([P, nh, D], F32, tag=f"OUT{nh}")
            h0 = hh
            q = 0
            while q < nh:
                step = 1 if nh == 1 else 2
                Y = psB.tile([P, step * D], F32, tag="Y", bufs=2)
                for t in range(step):
                    emit_hh(hh + t, Y, t * D)
                nc.vector.add_range_wrap(Y[:], Y[:], 0.0, 0.5, 1.0)
                nc.scalar.activation(OUT[:, q:q + step, :], Y[:], AF.Sin,
                                     bias=0.0, scale=2 * PI)
                hh += step
                q += step
            nc.sync.dma_start(out3[:, h0:h0 + nh, :], OUT[:])


def build_nc():
    nc = bacc.Bacc("TRN2", target_bir_lowering=False, debug=False,
                   num_devices=N_CORES)
    x = nc.dram_tensor("x", [S_PER], F32, kind="ExternalInput")
    sel_t = nc.dram_tensor("sel", [P, D], F16, kind="ExternalInput")
    bd_t = nc.dram_tensor("bd", [64, H * 32], F32, kind="ExternalInput")
    idn_t = nc.dram_tensor("idn", [P, P], F32, kind="ExternalInput")
    out = nc.dram_tensor("out", [S_PER, D], F32, kind="ExternalOutput")
    x2d = x.rearrange("(p h) -> p h", p=P)
    out3 = out.rearrange("(p h) d -> p h d", p=P)

    with tile.TileContext(nc) as tc:
        with (
            tc.tile_pool(name="sb", bufs=1) as sb,
            tc.tile_pool(name="outp", bufs=3) as out_pool,
        ):
            _emit_body(nc, tc, sb, out_pool, x2d, out3, sel_t[:, :],
                       bd_t[:, :], idn_t[:, :])

    nc.compile()
    return nc


_CACHED_NC = None


def kernel(input_ids, max_seq_len, d_model):
    """Full-input entry point: shards batch rows across the 8 cores."""
    global _CACHED_NC
    input_ids = np.ascontiguousarray(np.asarray(input_ids, dtype=np.float32))
    assert input_ids.shape == (B, S) and int(max_seq_len) == MAX_SEQ_LEN \
        and int(d_model) == D
    if _CACHED_NC is None:
        _CACHED_NC = build_nc()
    consts = host_constants()
    in_maps = [{"x": input_ids[c], **consts} for c in range(N_CORES)]
    res = run_bass_kernel_spmd(_CACHED_NC, in_maps,
                               core_ids=list(range(N_CORES)))
    return np.stack([res.results[c]["out"] for c in range(N_CORES)], axis=0)


# revision 8
# speedup vs baseline: 1.2527x; 1.0048x over previous
"""Chebyshev positional-embedding expansion kernel for Trainium2 (8 cores).

Computes out[b, s, d] = T_d(xhat[b, s]), xhat = 2*input_ids/max_seq_len - 1,
T_d = Chebyshev polynomial of the first kind, matching the jax.lax.scan
reference recurrence T_n = 2*xhat*T_{n-1} - T_{n-2} to ~1e-3 rel error.

Strategy (per core; batch row b == core id, no communication):
  T_d(cos t) = cos(d*t), so with phi = arccos(xhat)/(2pi) in [0, 0.5]:
      T[s, n] = cos(2pi * n * phi_s) = sin(2pi * cfrac(n*phi_s + 0.25))
  where cfrac(x) = x - round(x).  phi via the branch-free half-angle form
      arccos(x) = 2*atan(sqrt((1-x)/(1+x)))
  with the argument folded into [0, 1] by min/max + one reciprocal.
  Per position build 64-entry f16 tables
      c_b = cfrac(b*phi)    (b = n % 32),   g_a = cfrac(32a*phi)  (a = n//32)
  (phases via two K=32 PE matmuls against a host-provided block-diagonal
  ramp; cfrac via DVE round-to-nearest i32 cast + one fused
  scalar_tensor_tensor). One 2-byte DMA-xbar transpose turns the tables
  into per-hh [64, 128] lhsT blocks. Then per 128-position block and hh
  pair, K=64 f16 PE matmuls against a constant one-hot selector produce
      y[s, n] = c_{n%32} + g_{n//32} in [-1, 1]
  in PSUM; one DVE ADD_RANGE_WRAP custom op computes cfrac(y + 0.25) in
  [-0.5, 0.5]; one ACT Sin (scale=2pi) writes T to SBUF. Per output
  element: 1 f16 PE col-cycle + 1 DVE op + 1 ACT op -- every engine sits
  under the ~47us/core HBM write roofline, so the kernel is
  output-DMA bound.

Constants (one-hot selector, block-diag ramp, transpose identity) are
precomputed on the host and DMA'd in on the (otherwise idle-at-start)
sync queue; the input row rides the scalar HWDGE queue so it lands first.
Dummy f16 matmuls keep the PE HAM window busy through the prefix so the
array runs at 2.4 GHz when the real matmuls arrive.

Layout: s_local = p*32 + hh -> partition p owns 32 contiguous output rows.
"""

import numpy as np

import concourse.bacc as bacc
import concourse.mybir as mybir
from concourse import tile
from concourse.bass_utils import run_bass_kernel_spmd

F32 = mybir.dt.float32
F16 = mybir.dt.float16
I32 = mybir.dt.int32
OP = mybir.AluOpType
AF = mybir.ActivationFunctionType

N_CORES = 8
B, S, D = 8, 4096, 1024
MAX_SEQ_LEN = 4096
S_PER = B * S // N_CORES
P = 128
H = S_PER // P  # 32
PI = float(np.pi)

# output chunk sizes in hh units (sum = 32); first/last small so the DMA
# stream starts early and drains fast
CHUNKS = (1, 1, 2, 4, 4, 4, 4, 4, 4, 4)
WARM_PRE = 12   # PE warm-up matmuls before the table pipeline
WARM_MID = 6    # PE keep-warm matmuls during the cast/stt gap


def host_constants():
    n = np.arange(D)
    sel = np.zeros((P, D), np.float16)
    sel[0:32] = (n[None, :] % 32 == np.arange(32)[:, None])
    sel[32:64] = (n[None, :] // 32 == np.arange(32)[:, None])
    sel[64:128] = sel[0:64]
    bd = np.zeros((64, H * 32), np.float32)
    for k in range(32):
        bd[k, k * 32:(k + 1) * 32] = np.arange(32)
        bd[32 + k, k * 32:(k + 1) * 32] = np.arange(32)
    idn = np.eye(P, dtype=np.float32)
    return {"sel": sel, "bd": bd, "idn": idn}


def _emit_body(nc, tc, sb, out_pool, x2d, out3, sel_d, bd_d, idn_d):
    # ---------------- input (scalar HWDGE; lands first) ---------------
    X = sb.tile([P, H], F32, tag="X")
    nc.scalar.dma_start(X[:], x2d)

    # ---------------- constants via DMA (sync queue, idle early) ------
    SEL = sb.tile([P, D], F16, tag="SEL")
    BD = sb.tile([64, H * 32], F32, tag="BD")
    IDN = sb.tile([P, P], F32, tag="IDN")
    nc.sync.dma_start(SEL[:], sel_d)
    nc.sync.dma_start(BD[:], bd_d)
    nc.sync.dma_start(IDN[:], idn_d)

    # ---------------- phi = arccos(xhat)/(2pi), all-DVE ---------------
    XH = sb.tile([P, H], F32, tag="XH")
    U1 = sb.tile([P, H], F32, tag="U1")
    U2 = sb.tile([P, H], F32, tag="U2")
    MN = sb.tile([P, H], F32, tag="MN")
    MX = sb.tile([P, H], F32, tag="MX")
    RC = sb.tile([P, H], F32, tag="RC")
    Z2 = sb.tile([P, H], F32, tag="Z2")
    Z = sb.tile([P, H], F32, tag="Z")
    AT = sb.tile([P, H], F32, tag="AT")
    MM = sb.tile([P, H], F32, tag="MM")
    SG = sb.tile([P, H], F32, tag="SG")
    A0 = sb.tile([P, H], F32, tag="A0")
    F1 = sb.tile([P, H], F32, tag="F1")
    PHI = sb.tile([P, 64], F32, tag="PHI")  # cols 0:32 phi, 32:64 psi=32phi

    nc.vector.tensor_scalar(XH[:], X[:], 1.0 / (MAX_SEQ_LEN / 2), -1.0,
                            OP.mult, OP.add)
    nc.vector.tensor_scalar(U1[:], XH[:], -1.0, 1.0, OP.mult, OP.add)
    nc.vector.tensor_scalar(U2[:], XH[:], 1.0, 1e-30, OP.add, OP.max)
    nc.vector.tensor_tensor(MN[:], U1[:], U2[:], OP.min)
    nc.vector.tensor_tensor(MX[:], U1[:], U2[:], OP.max)
    nc.vector.reciprocal(RC[:], MX[:])
    nc.vector.tensor_tensor(Z2[:], MN[:], RC[:], OP.mult)
    nc.scalar.activation(Z[:], Z2[:], AF.Sqrt)
    nc.scalar.activation(AT[:], Z[:], AF.Arctan)
    # phi = (1-m)/2 + at*(2m-1)/pi,  m = (x >= 0)
    nc.vector.tensor_scalar(MM[:], XH[:], 0.0, None, OP.is_ge)
    nc.vector.tensor_scalar(SG[:], MM[:], 2.0 / PI, -1.0 / PI, OP.mult,
                            OP.add)
    nc.vector.tensor_scalar(A0[:], MM[:], -0.5, 0.5, OP.mult, OP.add)
    nc.vector.tensor_tensor(F1[:], AT[:], SG[:], OP.mult)
    nc.vector.tensor_tensor(PHI[:, 0:H], F1[:], A0[:], OP.add)
    nc.vector.tensor_scalar(PHI[:, H:2 * H], PHI[:, 0:H], 32.0, None,
                            OP.mult)

    # ---------------- tables (f16) ----------------
    TBL = sb.tile([P, H, 64], F16, tag="TBL")
    TBLT = sb.tile([P, 16, P], F16, tag="TBLT")

    with tc.tile_pool(name="psA", bufs=1, space="PSUM") as psA:
        # PE warm-up: dummy f16 matmuls as soon as SEL lands
        W = psA.tile([P, 512], F32, tag="WARM")
        for _ in range(WARM_PRE):
            nc.tensor.matmul(W[:], SEL[0:64, 0:P], SEL[0:64, 0:512])

        PHT_PS = psA.tile([64, P], F32, tag="PHT_PS")
        nc.tensor.transpose(PHT_PS[:], PHI[:], IDN[:])
        PHT = sb.tile([64, P], F32, tag="PHT")
        nc.scalar.activation(PHT[:], PHT_PS[:], AF.Copy)

        # phases: YCG[:, 0:1024] = c-part, [:, 1024:2048] = g-part
        YCG = psA.tile([P, 2048], F32, tag="YCG")
        for j in (0, 1):
            nc.tensor.matmul(YCG[:, j * 512:(j + 1) * 512], PHT[0:32, :],
                             BD[0:32, j * 512:(j + 1) * 512])
            nc.tensor.matmul(YCG[:, 1024 + j * 512:1024 + (j + 1) * 512],
                             PHT[32:64, :], BD[32:64, j * 512:(j + 1) * 512])

        # keep PE hot through the cast/stt gap
        for _ in range(WARM_MID):
            nc.tensor.matmul(W[:], SEL[0:64, 0:P], SEL[0:64, 0:512])

        # table = y - round(y)  (round via i32 cast, round-to-nearest)
        KK = sb.tile([P, 2048], I32, tag="KK")
        nc.vector.tensor_copy(KK[:], YCG[:])
        # interleave c/g per hh: TBL[p, h, u] with u<32 from c, u>=32 from g
        YC3 = YCG[:, 0:1024].rearrange("p (h b) -> p h b", h=H)
        YG3 = YCG[:, 1024:2048].rearrange("p (h a) -> p h a", h=H)
        KC3 = KK[:, 0:1024].rearrange("p (h b) -> p h b", h=H)
        KG3 = KK[:, 1024:2048].rearrange("p (h a) -> p h a", h=H)
        nc.vector.scalar_tensor_tensor(TBL[:, :, 0:32], YC3, 0.0, KC3,
                                       OP.subtract, OP.subtract)
        nc.vector.scalar_tensor_tensor(TBL[:, :, 32:64], YG3, 0.0, KG3,
                                       OP.subtract, OP.subtract)

    # one 2-byte xbar transpose: TBLT[p, j, q] = TBL2[q, j*128+p]
    TBL2 = TBL[:].rearrange("p h u -> p (h u)")
    nc.scalar.dma_start_transpose(TBLT[:], TBL2)

    # ---------------- big loop: hh pairs ----------------
    with tc.tile_pool(name="psB", bufs=1, space="PSUM") as psB:

        def emit_hh(hh, Y, yoff):
            lhsT = TBLT[(hh % 2) * 64:(hh % 2) * 64 + 64, hh // 2, :]
            rb = (hh % 2) * 64
            nc.tensor.matmul(Y[:, yoff:yoff + 512], lhsT,
                             SEL[rb:rb + 64, 0:512])
            nc.tensor.matmul(Y[:, yoff + 512:yoff + 1024], lhsT,
                             SEL[rb:rb + 64, 512:1024])

        hh = 0
        for nh in CHUNKS:
            OUT = out_pool.tile([P, nh, D], F32, tag=f"OUT{nh}")
            h0 = hh
            q = 0
            while q < nh:
                step = 1 if nh == 1 else 2
                Y = psB.tile([P, step * D], F32, tag="Y", bufs=2)
                for t in range(step):
                    emit_hh(hh + t, Y, t * D)
                # cfrac(y + 0.25): wrap handles the cos->sin quarter turn
                nc.vector.add_range_wrap(Y[:], Y[:], 0.25, 0.5, 1.0)
                nc.scalar.activation(OUT[:, q:q + step, :], Y[:], AF.Sin,
                                     bias=0.0, scale=2 * PI)
                hh += step
                q += step
            nc.sync.dma_start(out3[:, h0:h0 + nh, :], OUT[:])


def build_nc():
    nc = bacc.Bacc("TRN2", target_bir_lowering=False, debug=False,
                   num_devices=N_CORES)
    x = nc.dram_tensor("x", [S_PER], F32, kind="ExternalInput")
    sel_t = nc.dram_tensor("sel", [P, D], F16, kind="ExternalInput")
    bd_t = nc.dram_tensor("bd", [64, H * 32], F32, kind="ExternalInput")
    idn_t = nc.dram_tensor("idn", [P, P], F32, kind="ExternalInput")
    out = nc.dram_tensor("out", [S_PER, D], F32, kind="ExternalOutput")
    x2d = x.rearrange("(p h) -> p h", p=P)
    out3 = out.rearrange("(p h) d -> p h d", p=P)

    with tile.TileContext(nc) as tc:
        with (
            tc.tile_pool(name="sb", bufs=1) as sb,
            tc.tile_pool(name="outp", bufs=3) as out_pool,
        ):
            _emit_body(nc, tc, sb, out_pool, x2d, out3, sel_t[:, :],
                       bd_t[:, :], idn_t[:, :])

    nc.compile()
    return nc


_CACHED_NC = None


def kernel(input_ids, max_seq_len, d_model):
    """Full-input entry point: shards batch rows across the 8 cores."""
    global _CACHED_NC
    input_ids = np.ascontiguousarray(np.asarray(input_ids, dtype=np.float32))
    assert input_ids.shape == (B, S) and int(max_seq_len) == MAX_SEQ_LEN \
        and int(d_model) == D
    if _CACHED_NC is None:
        _CACHED_NC = build_nc()
    consts = host_constants()
    in_maps = [{"x": input_ids[c], **consts} for c in range(N_CORES)]
    res = run_bass_kernel_spmd(_CACHED_NC, in_maps,
                               core_ids=list(range(N_CORES)))
    return np.stack([res.results[c]["out"] for c in range(N_CORES)], axis=0)
